# revision 24
# baseline (speedup 1.0000x reference)
"""GNN message-passing kernel for Trainium2 (8 NeuronCores, batch-sharded).

Computes, for each batch b:
    neigh[i, d] = max_j (A[b, j, i] * x[b, j, d])      (== reference masked max)
    out = x @ W_self.T + neigh @ W_neigh.T

Algorithm per batch (exact on {0,1} adjacency with at least one non-neighbor
per node, which the reference's where(...,0).max() semantics imply):
  - x^T and A^T built with PE transposes (identity matmul).
  - A^T mapped to additive penalties BIG*(A^T - 1) in {0, -BIG}, flattened
    into one SBUF partition.
  - Per group of 6 targets (two PSUM banks, 3 slots each): PE assembles
    x^T + penalty_i in PSUM (transpose-matmul x-fills + K=1 ones-matmul
    penalty broadcast, bf16 where exact), then one grouped 4D-AP DVE
    tensor_reduce computes max over j for all 6 targets in a single op.
  - neigh = relu(that max); final projections as two PSUM-accumulated matmuls.
"""

import numpy as np

import concourse.bacc as bacc
import concourse.bass as bass
import concourse.mybir as mybir
import concourse.tile as tile
from concourse.bass_utils import run_bass_kernel_spmd
from concourse.masks import make_identity

B, S, D = 32, 150, 128
NCORES = 8
BPC = B // NCORES  # batches per core
BIG = 1024.0  # penalty scale; |x| < 6 so 1024 dominates and stays exact in f32
GRP = 6  # targets per PSUM tile (two banks; 3 slots of 150 per 512-wide bank)
BANK = 512  # fp32 elements per PSUM bank partition

f32 = mybir.dt.float32
bf16 = mybir.dt.bfloat16
i32 = mybir.dt.int32

_PROGRAM_CACHE: dict[str, bass.Bass] = {}


def _build_batch(nc, tc, cpool, wpool, ppool, mbpool, consts, x_d, a_d, out_d, b):
    ident, ident_bf, ones1, wst_sb, wnt_sb = consts

    # ---- load x (2 j-chunks)
    x0 = wpool.tile([128, D], f32, tag="x0")
    x1 = wpool.tile([22, D], f32, tag="x1")
    nc.sync.dma_start(x0[:], x_d[b, 0:128, :])
    nc.sync.dma_start(x1[:], x_d[b, 128:150, :])

    # ---- xT = x^T [D, S] via PE transpose
    xT_ps = ppool.tile([D, S], f32, tag="tps")
    nc.tensor.transpose(xT_ps[:, 0:128], x0[:], ident[:])
    nc.tensor.transpose(xT_ps[:, 128:150], x1[:], ident[0:22, 0:22])
    xT = wpool.tile([D, S], f32, tag="xT_sb")
    nc.scalar.copy(xT[:], xT_ps[:])

    # ---- load A int32 (2 j-chunks), cast to bf16 on DVE ({0,1}: exact)
    a0_i = wpool.tile([128, S], i32, tag="a0i")
    a1_i = wpool.tile([22, S], i32, tag="a1i")
    nc.sync.dma_start(a0_i[:], a_d[b, 0:128, :])
    nc.sync.dma_start(a1_i[:], a_d[b, 128:150, :])
    a0 = wpool.tile([128, S], bf16, tag="a0")
    a1 = wpool.tile([22, S], bf16, tag="a1")
    nc.vector.tensor_copy(a0[:], a0_i[:])
    nc.vector.tensor_copy(a1[:], a1_i[:])

    # ---- A^T via 4 bf16 PE transposes, then penalty BIG*(A^T - 1) in bf16
    at0_ps = ppool.tile([128, S], bf16, tag="tps")
    nc.tensor.transpose(at0_ps[:, 0:128], a0[:, 0:128], ident_bf[:])
    nc.tensor.transpose(at0_ps[:, 128:150], a1[:, 0:128], ident_bf[0:22, 0:22])
    pen0 = wpool.tile([128, S], bf16, tag="pen0")
    nc.scalar.activation(
        pen0[:], at0_ps[:], mybir.ActivationFunctionType.Copy, bias=-BIG, scale=BIG
    )
    at1_ps = ppool.tile([22, S], bf16, tag="tps")
    nc.tensor.transpose(at1_ps[:, 0:128], a0[:, 128:150], ident_bf[:])
    nc.tensor.transpose(at1_ps[:, 128:150], a1[:, 128:150], ident_bf[0:22, 0:22])
    pen1 = wpool.tile([22, S], bf16, tag="pen1")
    nc.scalar.activation(
        pen1[:], at1_ps[:], mybir.ActivationFunctionType.Copy, bias=-BIG, scale=BIG
    )

    # ---- flatten penalties into one partition: pflat[0, i*S + j]  (bf16)
    pflat = wpool.tile([1, S * S], bf16, tag="pflat")
    nc.sync.dma_start(pflat[0:1, 0 : 128 * S], pen0[:, :])
    nc.sync.dma_start(pflat[0:1, 128 * S : S * S], pen1[:, :])

    # ---- masked max per group of GRP targets: reduce_max_j (xT + penalty_i)
    # Multi-bank PSUM tile; HALF slots of S columns per 512-wide bank.
    HALF = BANK // S
    NBANK = GRP // HALF
    rT = wpool.tile([D, S], f32, tag="rT")
    for i0 in range(0, S, GRP):
        g = min(GRP, S - i0)
        nbank = (g + HALF - 1) // HALF
        mb = mbpool.tile([D, NBANK * BANK], f32, tag="mb")
        # penalty broadcast opens each bank's accumulation group
        for nb in range(nbank):
            lo_i = i0 + nb * HALF
            hi_i = min(i0 + (nb + 1) * HALF, i0 + g)
            nc.tensor.matmul(
                mb[:, nb * BANK : nb * BANK + (hi_i - lo_i) * S],
                ones1[:],
                pflat[0:1, lo_i * S : hi_i * S],
                start=True,
                stop=False,
            )
        # x-fill: transpose-matmuls accumulate x^T into each slot
        for c in range(g):
            base = (c // HALF) * BANK + (c % HALF) * S
            last = c % HALF == HALF - 1 or c == g - 1  # closes this bank
            nc.tensor.matmul(
                mb[:, base : base + 128],
                x0[:],
                ident[:],
                is_transpose=True,
                start=False,
                stop=False,
            )
            nc.tensor.matmul(
                mb[:, base + 128 : base + 150],
                x1[:],
                ident[0:22, 0:22],
                is_transpose=True,
                start=False,
                stop=last,
            )
        if g == GRP:
            red_in = (
                mb[:]
                .rearrange("p (b r) -> p b r", b=NBANK)[:, :, 0 : HALF * S]
                .rearrange("p b (g s) -> p b g s", g=HALF)
            )
            nc.vector.tensor_reduce(
                out=rT[:, i0 : i0 + GRP],
                in_=red_in,
                axis=mybir.AxisListType.X,
                op=mybir.AluOpType.max,
            )
        else:
            for nb in range(nbank):
                lo_i = i0 + nb * HALF
                hi_i = min(i0 + (nb + 1) * HALF, i0 + g)
                red_in = mb[:, nb * BANK : nb * BANK + (hi_i - lo_i) * S].rearrange(
                    "p (g s) -> p g s", g=hi_i - lo_i
                )
                nc.vector.tensor_reduce(
                    out=rT[:, lo_i:hi_i],
                    in_=red_in,
                    axis=mybir.AxisListType.X,
                    op=mybir.AluOpType.max,
                )

    # ---- neigh^T = relu(rT)
    rT_relu = wpool.tile([D, S], f32, tag="rTrelu")
    nc.scalar.activation(rT_relu[:], rT[:], mybir.ActivationFunctionType.Relu)

    # ---- out = x @ Ws^T + neigh @ Wn^T   (contract d; out [s-chunk, e])
    for c, (lo, hi) in enumerate([(0, 128), (128, 150)]):
        m = hi - lo
        o_ps = ppool.tile([m, D], f32, tag="wtops")
        nc.tensor.matmul(o_ps[:], xT[:, lo:hi], wst_sb[:], start=True, stop=False)
        nc.tensor.matmul(o_ps[:], rT_relu[:, lo:hi], wnt_sb[:], start=False, stop=True)
        o_sb = wpool.tile([m, D], f32, tag=f"osb{c}")
        nc.scalar.copy(o_sb[:], o_ps[:])
        nc.sync.dma_start(out_d[b, lo:hi, :], o_sb[:])


def _build_program() -> bass.Bass:
    if "nc" in _PROGRAM_CACHE:
        return _PROGRAM_CACHE["nc"]

    nc = bacc.Bacc("TRN2", target_bir_lowering=False, debug=False)
    x_d = nc.dram_tensor("x", [BPC, S, D], f32, kind="ExternalInput").ap()
    a_d = nc.dram_tensor("A", [BPC, S, S], i32, kind="ExternalInput").ap()
    ws_d = nc.dram_tensor("ws", [D, D], f32, kind="ExternalInput").ap()
    wn_d = nc.dram_tensor("wn", [D, D], f32, kind="ExternalInput").ap()
    out_d = nc.dram_tensor("out", [BPC, S, D], f32, kind="ExternalOutput").ap()

    with tile.TileContext(nc) as tc:
        with (
            tc.tile_pool(name="const", bufs=1) as cpool,
            tc.tile_pool(name="work", bufs=3) as wpool,
            tc.tile_pool(name="psum", bufs=1, space="PSUM") as ppool,
            tc.tile_pool(name="psum_mb", bufs=3, space="PSUM") as mbpool,
        ):
            ident = cpool.tile([128, 128], f32)
            make_identity(nc, ident[:])
            ident_bf = cpool.tile([128, 128], bf16, tag="identbf")
            nc.vector.tensor_copy(ident_bf[:], ident[:])
            ones1 = cpool.tile([1, 128], bf16, tag="ones1")
            nc.gpsimd.memset(ones1[:], 1.0)

            ws_sb = cpool.tile([D, D], f32, tag="ws")
            wn_sb = cpool.tile([D, D], f32, tag="wn")
            nc.sync.dma_start(ws_sb[:], ws_d[:, :])
            nc.sync.dma_start(wn_sb[:], wn_d[:, :])
            wst_sb = cpool.tile([D, D], f32, tag="wst")
            wnt_sb = cpool.tile([D, D], f32, tag="wnt")
            wt_ps = ppool.tile([D, D], f32, tag="wtops")
            nc.tensor.transpose(wt_ps[:], ws_sb[:], ident[:])
            nc.scalar.copy(wst_sb[:], wt_ps[:])
            wt_ps2 = ppool.tile([D, D], f32, tag="wtops")
            nc.tensor.transpose(wt_ps2[:], wn_sb[:], ident[:])
            nc.scalar.copy(wnt_sb[:], wt_ps2[:])

            consts = (ident, ident_bf, ones1, wst_sb, wnt_sb)
            for b in range(BPC):
                _build_batch(
                    nc, tc, cpool, wpool, ppool, mbpool, consts, x_d, a_d, out_d, b
                )

    nc.compile()
    _PROGRAM_CACHE["nc"] = nc
    return nc


def kernel(x, A, W_self, W_neigh, **kwargs):
    x = np.ascontiguousarray(np.asarray(x, dtype=np.float32))
    A = np.ascontiguousarray(np.asarray(A, dtype=np.int32))
    W_self = np.ascontiguousarray(np.asarray(W_self, dtype=np.float32))
    W_neigh = np.ascontiguousarray(np.asarray(W_neigh, dtype=np.float32))

    nc = _build_program()
    in_maps = [
        {
            "x": x[c * BPC : (c + 1) * BPC],
            "A": A[c * BPC : (c + 1) * BPC],
            "ws": W_self,
            "wn": W_neigh,
        }
        for c in range(NCORES)
    ]
    res = run_bass_kernel_spmd(nc, in_maps, core_ids=list(range(NCORES)), **kwargs)
    out = np.concatenate([res.results[c]["out"] for c in range(NCORES)], axis=0)
    return np.ascontiguousarray(out.astype(np.float32))
